# revision 46
# baseline (speedup 1.0000x reference)
"""Trainium2 Bass kernel for nn_DualChannelTransformer.

Sharding: 8 cores = 4 batches x 2 channels (left/right). Each core runs one
channel's transformer stack. Cross-attention K/V activations are swapped
between channel pairs once per layer with a single ReduceScatter (own slot
zeroed, fp8 payload, output = partner's x only -- half an AllGather's
traffic and one 15us collective constant instead of two). Layer 0 needs no
collective: each core also computes the partner channel's input projection
locally (the wrist inputs are tiny).

Compute layout: activations transposed (xT [D, S]); heavy matmuls run in
fp8e4 with DoubleRow perf mode (2 contraction tiles of 128 per instruction).
Weights are pre-scaled x64 so fp8 quantization stays in the normal range;
descales fold into evacuation ops / the exp scale. Residual adds ride the PE
via scaled-identity matmuls so PSUM evacuations stay single-op ScalarE
Copy/Relu activations (zero-bias fast path; a general biased path is kept
for robustness). Attention: scoresT [k, q] in bf16 (Q/K held at 64x scale),
exp on ScalarE with scale 1/(8*4096) writing fp8, fused ctx+denominator
DoubleRow matmul over st-pairs (V8 with a ones column, M=65). LayerNorm over
D uses all-ones bf16 matmuls for mean/E[x^2], Abs_reciprocal_sqrt for the
inverse std, and dual-emits bf16 x (gpsimd) + fp8 x8 (DVE) so the fp8
consumers are unblocked first.
"""

import os
import sys

import numpy as np

for _p in ("/opt/trn_rl_repo", "/root/.axon_site/_ro/trn_rl_repo"):
    if os.path.isdir(_p) and _p not in sys.path:
        sys.path.insert(0, _p)

import ml_dtypes

import concourse.bass as bass
import concourse.tile as tile
from concourse import bacc, mybir
from concourse.bass import ds
from concourse.bass_utils import run_bass_kernel_spmd

F32 = mybir.dt.float32
F32R = mybir.dt.float32r
BF16 = mybir.dt.bfloat16
FP8 = mybir.dt.float8e4
AF = mybir.ActivationFunctionType
OP = mybir.AluOpType
DR = mybir.MatmulPerfMode.DoubleRow
BF = ml_dtypes.bfloat16
F8 = ml_dtypes.float8_e4m3fn

B, S, IN, D, H, LAYERS, F = 4, 1024, 6, 512, 8, 4, 2048
DK = D // H
EPS = 1e-5
NCORES = 8
DC = D // 128   # 4 activation partition chunks
KP = DC // 2    # 2 kc pairs
FC = F // 128   # 16
NQ = S // 512   # 2 moving-dim chunks
KT8 = S // 128  # 8 k tiles
WS = 64.0       # fp8 weight scale
VP = 72         # padded DK+1 for vext (16B-aligned pair stride)

_CACHE = {}


def _emit_ln(nc, pools, a_t, g_sb, b_sb, ln_i, zb, need_x8=True):
    """LayerNorm over D (partition axis) of a_t [128,4,1024] bf16.

    Returns (x [128,4,1024] bf16, [x8h0, x8h1] fp8 half tiles [128,4,512]).
    """
    act, pbig, xpool, consts = (pools["act"], pools["pbig"], pools["xpool"],
                                pools["consts"])
    oavg_bf = consts["oavg_bf"]    # [128,128] bf16 = 1/512
    eps_col = consts["eps_col"]    # [128,1] f32 = EPS

    out = xpool.tile([128, DC, 1024], BF16, tag="x")
    x8h = [xpool.tile([128, DC, 512], FP8, tag=f"x8h{h}", name=f"x8h{h}")
           for h in range(NQ)] if need_x8 else [None, None]
    for nq in range(NQ):
        s0 = nq * 512
        sq = act.tile([128, DC, 512], BF16, tag="sq", bufs=2)
        for kc in range(DC):
            nc.vector.tensor_tensor(out=sq[:, kc, :],
                                    in0=a_t[:, kc, s0:s0 + 512],
                                    in1=a_t[:, kc, s0:s0 + 512], op=OP.mult)
        mps = pbig.tile([128, 1024], F32, tag="big")
        for kc in range(DC):
            nc.tensor.matmul(mps[:, 0:512], oavg_bf[:],
                             a_t[:, kc, s0:s0 + 512],
                             start=(kc == 0), stop=(kc == DC - 1))
        for kc in range(DC):
            nc.tensor.matmul(mps[:, 512:1024], oavg_bf[:],
                             sq[:, kc, :],
                             start=(kc == 0), stop=(kc == DC - 1))
        m2 = act.tile([128, 512], BF16, tag="lnt", bufs=3)
        nc.scalar.activation(out=m2[:], in_=mps[:, 0:512], func=AF.Square)
        work = act.tile([128, 512], BF16, tag="lnt2", bufs=3)
        nc.vector.tensor_tensor(out=work[:], in0=mps[:, 512:1024], in1=m2[:],
                                op=OP.subtract)
        # inv_std in one ACT op (arg is var+eps > 0, so abs is a no-op)
        nc.scalar.activation(out=work[:], in_=work[:],
                             func=AF.Abs_reciprocal_sqrt, bias=eps_col[:])
        for dc in range(DC):
            am = act.tile([128, 512], BF16, tag="am", bufs=4)
            nc.vector.tensor_tensor(out=am[:], in0=a_t[:, dc, s0:s0 + 512],
                                    in1=mps[:, 0:512], op=OP.subtract)
            if zb:
                # dual-emit: fp8 shadow on DVE (feeds next block's matmuls
                # first), bf16 canonical on gpsimd (residual, needed later);
                # final layer (no x8 consumer) takes the fast DVE path only
                nc.vector.scalar_tensor_tensor(
                    out=(x8h[nq][:, dc, :] if need_x8
                         else out[:, dc, s0:s0 + 512]),
                    in0=am[:], scalar=g_sb[:, ln_i, dc:dc + 1], in1=work[:],
                    op0=OP.mult, op1=OP.mult)
                if need_x8:
                    gt = act.tile([128, 512], BF16, tag="gt", bufs=4)
                    nc.gpsimd.tensor_scalar_mul(
                        out=gt[:], in0=am[:],
                        scalar1=g_sb[:, ln_i, dc:dc + 1])
                    nc.gpsimd.tensor_tensor(out=out[:, dc, s0:s0 + 512],
                                            in0=gt[:], in1=work[:],
                                            op=OP.mult)
            else:
                nc.vector.scalar_tensor_tensor(
                    out=am[:], in0=am[:], scalar=g_sb[:, ln_i, dc:dc + 1],
                    in1=work[:], op0=OP.mult, op1=OP.mult)
                nc.gpsimd.tensor_scalar_add(out=out[:, dc, s0:s0 + 512],
                                            in0=am[:],
                                            scalar1=b_sb[:, ln_i, dc:dc + 1])
                if need_x8:
                    nc.gpsimd.tensor_copy(out=x8h[nq][:, dc, :],
                                          in_=out[:, dc, s0:s0 + 512])
    return out, x8h


def _emit_attn(nc, pools, dram, x_q, xq8, xkv8, li, bi, zb):
    """One attention block.

    x_q: [128,4,1024] bf16 (residual); xq8/xkv8: fp8 half-tile lists.
    Returns (x, x8h) from the post-LN.
    """
    act, pbig, pctx, consts = (pools["act"], pools["pbig"], pools["pctx"],
                               pools["consts"])
    ones_row = consts["ones_row"]      # [1,128] bf16 ones
    ident64 = consts["ident64"]        # [128,128] bf16 64*I

    wqkv = pools["w"].tile([128, 16, 512], FP8, tag="wbig")
    nc.scalar.dma_start(wqkv[:], dram["qkv_w"][li, bi])
    bqko = vb = None
    if not zb:
        bqko = act.tile([128, 3, 4], F32, tag="bqko", bufs=2)
        nc.scalar.dma_start(bqko[:], dram["qko_b"][li, bi])
        vb = act.tile([1, 512], BF16, tag="vb", bufs=2)
        nc.scalar.dma_start(vb[:], dram["v_b"][li, bi])

    # ---- Q/K projections (transposed, bf16 out held at 64x scale) ----
    QT = act.tile([128, DC, 1024], BF16, tag="qt")
    KT = act.tile([128, DC, 1024], BF16, tag="kt")
    for pi, (dst, src) in enumerate(((QT, xq8), (KT, xkv8))):
        for nq in range(NQ):
            for mc in range(DC):
                ps = pbig.tile([128, 1024], F32, tag="big")
                for kp in range(KP):
                    nc.tensor.matmul(
                        ps[:, 0:512],
                        wqkv[:, pi * 4 + 2 * kp:pi * 4 + 2 * kp + 2,
                             mc * 128:(mc + 1) * 128],
                        src[nq][:, 2 * kp:2 * kp + 2, :],
                        start=(kp == 0), stop=(kp == KP - 1), perf_mode=DR)
                if zb:
                    nc.scalar.activation(
                        out=dst[:, mc, nq * 512:(nq + 1) * 512],
                        in_=ps[:, 0:512], func=AF.Copy)
                else:
                    nc.vector.tensor_scalar_add(
                        out=dst[:, mc, nq * 512:(nq + 1) * 512],
                        in0=ps[:, 0:512], scalar1=bqko[:, pi, mc:mc + 1])

    # ---- V projection + ones column for the denominator ----
    vext = []
    for sp in range(KT8 // 2):
        vt = pools["vext"].tile([128, 2, H, VP], FP8, tag=f"v{sp}",
                                name=f"v{sp}")
        vext.append(vt)
    for st in range(KT8):
        sp, stl = st // 2, st % 2
        hh, loc = st // 4, st % 4
        ps = pbig.tile([128, 1024], F32, tag="big")
        for kp in range(KP):
            nc.tensor.matmul(ps[:, 0:512],
                             xkv8[hh][:, 2 * kp:2 * kp + 2,
                                      loc * 128:(loc + 1) * 128],
                             wqkv[:, 8 + 2 * kp:10 + 2 * kp, :],
                             start=(kp == 0),
                             stop=(zb and kp == KP - 1), perf_mode=DR)
        if not zb:
            nc.tensor.matmul(ps[:, 0:512], ones_row[:], vb[:],
                             start=False, stop=True)
        if zb:
            nc.scalar.activation(
                out=vext[sp][:, stl, :, 0:DK],
                in_=ps[:, 0:512].rearrange("p (h k) -> p h k", h=H),
                func=AF.Copy, scale=float(1.0 / WS))
        else:
            nc.vector.tensor_scalar_mul(
                out=vext[sp][:, stl, :, 0:DK],
                in0=ps[:, 0:512].rearrange("p (h k) -> p h k", h=H),
                scalar1=1.0 / WS)
        nc.vector.memset(vext[sp][:, stl, :, DK:DK + 1], 1.0)

    # ---- attention core: scoresT -> exp(fp8) -> fused ctx+denom (DR) ----
    ctxT8 = act.tile([128, DC, 1024], FP8, tag="ctxT")
    for h in range(H):
        hp, hr = h // 2, (h % 2) * 64
        cps = pctx.tile([DK + 1, 1024], F32, tag="ctx")
        for sp in range(KT8 // 2):
            exp8 = act.tile([128, 2, 1024], FP8, tag="exp", bufs=6)
            for stl in range(2):
                st = sp * 2 + stl
                sps = pbig.tile([128, 1024], F32, tag="big")
                for nq in range(NQ):
                    nc.tensor.matmul(sps[:, nq * 512:(nq + 1) * 512],
                                     KT[hr:hr + 64, hp, st * 128:(st + 1) * 128],
                                     QT[hr:hr + 64, hp, nq * 512:(nq + 1) * 512],
                                     start=True, stop=True)
                nc.scalar.activation(out=exp8[:, stl, :], in_=sps[:],
                                     func=AF.Exp,
                                     scale=float(1.0 / (np.sqrt(DK) * WS * WS)))
            for nq in range(NQ):
                nc.tensor.matmul(cps[:, nq * 512:(nq + 1) * 512],
                                 vext[sp][:, :, h, 0:DK + 1],
                                 exp8[:, :, nq * 512:(nq + 1) * 512],
                                 start=(sp == 0), stop=(sp == KT8 // 2 - 1),
                                 perf_mode=DR)
        rd = act.tile([1, 1024], BF16, tag="rd", bufs=2)
        nc.vector.reciprocal(out=rd[:], in_=cps[DK:DK + 1, :])
        rdb = act.tile([64, 1024], BF16, tag="rdb", bufs=2)
        nc.gpsimd.partition_broadcast(out_ap=rdb[:], in_ap=rd[:])
        nc.vector.tensor_tensor(out=ctxT8[hr:hr + 64, hp, :], in0=cps[0:DK, :],
                                in1=rdb[:], op=OP.mult)

    # ---- O projection (DR) + bias + residual via 64*I matmul ----
    a_t = act.tile([128, DC, 1024], BF16, tag="a", bufs=2)
    for mc in range(DC):
        ps = pbig.tile([128, 1024], F32, tag="big")
        for nq in range(NQ):
            for kp in range(KP):
                nc.tensor.matmul(ps[:, nq * 512:(nq + 1) * 512],
                                 wqkv[:, 12 + 2 * kp:14 + 2 * kp,
                                      mc * 128:(mc + 1) * 128],
                                 ctxT8[:, 2 * kp:2 * kp + 2,
                                       nq * 512:(nq + 1) * 512],
                                 start=(kp == 0), stop=False, perf_mode=DR)
            nc.tensor.matmul(ps[:, nq * 512:(nq + 1) * 512], ident64[:],
                             x_q[:, mc, nq * 512:(nq + 1) * 512],
                             start=False, stop=True)
        if zb:
            nc.scalar.activation(out=a_t[:, mc, :], in_=ps[:], func=AF.Copy,
                                 scale=float(1.0 / WS))
        else:
            nc.vector.tensor_scalar(out=a_t[:, mc, :], in0=ps[:],
                                    scalar1=1.0 / WS,
                                    scalar2=bqko[:, 2, mc:mc + 1],
                                    op0=OP.mult, op1=OP.add)
    return _emit_ln(nc, pools, a_t, pools["lng_sb"][li], pools["lnb_sb"][li],
                    bi, zb)


def _emit_ffn(nc, pools, dram, x_t, x8h, li, zb, need_x8=True):
    act, pbig = pools["act"], pools["pbig"]
    ident4k = pools["consts"]["ident4k"]   # [128,128] bf16 4096*I
    w1 = pools["w"].tile([128, 4, 2048], FP8, tag="wbig")
    nc.scalar.dma_start(w1[:], dram["ff_w1"][li])
    w2 = pools["w"].tile([128, 16, 512], FP8, tag="wbig")
    nc.scalar.dma_start(w2[:], dram["ff_w2"][li])
    b1 = b2 = None
    if not zb:
        b1 = act.tile([128, 16], F32, tag="b1", bufs=2)
        nc.scalar.dma_start(b1[:], dram["ff_b1"][li])
        b2 = act.tile([128, 4], F32, tag="b2", bufs=2)
        nc.scalar.dma_start(b2[:], dram["ff_b2"][li])

    a_t = act.tile([128, DC, 1024], BF16, tag="a", bufs=2)
    for half in range(2):
        s0 = half * 512
        hT8 = act.tile([128, 16, 512], FP8, tag="hT", bufs=2)
        for mf in range(FC):
            ps = pbig.tile([128, 1024], F32, tag="big")
            for kp in range(KP):
                nc.tensor.matmul(ps[:, 0:512],
                                 w1[:, 2 * kp:2 * kp + 2,
                                    mf * 128:(mf + 1) * 128],
                                 x8h[half][:, 2 * kp:2 * kp + 2, :],
                                 start=(kp == 0), stop=(kp == KP - 1),
                                 perf_mode=DR)
            if zb:
                nc.scalar.activation(out=hT8[:, mf, :], in_=ps[:, 0:512],
                                     func=AF.Relu)
            else:
                nc.vector.tensor_scalar(out=hT8[:, mf, :], in0=ps[:, 0:512],
                                        scalar1=b1[:, mf:mf + 1], scalar2=0.0,
                                        op0=OP.add, op1=OP.max)
        for mc in range(DC):
            ps = pbig.tile([128, 1024], F32, tag="big")
            for kp in range(FC // 2):
                nc.tensor.matmul(ps[:, 0:512],
                                 w2[:, 2 * kp:2 * kp + 2,
                                    mc * 128:(mc + 1) * 128],
                                 hT8[:, 2 * kp:2 * kp + 2, :],
                                 start=(kp == 0), stop=False, perf_mode=DR)
            nc.tensor.matmul(ps[:, 0:512], ident4k[:],
                             x_t[:, mc, s0:s0 + 512],
                             start=False, stop=True)
            if zb:
                nc.scalar.activation(out=a_t[:, mc, s0:s0 + 512],
                                     in_=ps[:, 0:512], func=AF.Copy,
                                     scale=float(1.0 / (WS * WS)))
            else:
                nc.vector.tensor_scalar(out=a_t[:, mc, s0:s0 + 512],
                                        in0=ps[:, 0:512],
                                        scalar1=1.0 / (WS * WS),
                                        scalar2=b2[:, mc:mc + 1],
                                        op0=OP.mult, op1=OP.add)
    return _emit_ln(nc, pools, a_t, pools["lng_sb"][li], pools["lnb_sb"][li],
                    2, zb, need_x8)


def _build(n_layers=LAYERS, zb=True):
    nc = bacc.Bacc("TRN2", target_bir_lowering=False, debug=False,
                   num_devices=NCORES)

    dram = {}
    dram["wT"] = nc.dram_tensor("wT", [2, IN, S], F32R, kind="ExternalInput")
    dram["w_in"] = nc.dram_tensor("w_in", [2, IN, D], F32R,
                                  kind="ExternalInput")
    dram["b_in"] = nc.dram_tensor("b_in", [2, 128, DC], F32,
                                  kind="ExternalInput")
    dram["peT"] = nc.dram_tensor("peT", [128, DC, S], BF16,
                                 kind="ExternalInput")
    dram["qkv_w"] = nc.dram_tensor("qkv_w", [LAYERS, 2, 128, 16, 512], FP8,
                                   kind="ExternalInput")
    dram["qko_b"] = nc.dram_tensor("qko_b", [LAYERS, 2, 128, 3, 4], F32,
                                   kind="ExternalInput")
    dram["v_b"] = nc.dram_tensor("v_b", [LAYERS, 2, 1, 512], BF16,
                                 kind="ExternalInput")
    dram["ln_g"] = nc.dram_tensor("ln_g", [128, LAYERS, 3, 4], F32,
                                  kind="ExternalInput")
    dram["ln_b"] = nc.dram_tensor("ln_b", [128, LAYERS, 3, 4], F32,
                                  kind="ExternalInput")
    dram["ff_w1"] = nc.dram_tensor("ff_w1", [LAYERS, 128, 4, 2048], FP8,
                                   kind="ExternalInput")
    dram["ff_b1"] = nc.dram_tensor("ff_b1", [LAYERS, 128, 16], F32,
                                   kind="ExternalInput")
    dram["ff_w2"] = nc.dram_tensor("ff_w2", [LAYERS, 128, 16, 512], FP8,
                                   kind="ExternalInput")
    dram["ff_b2"] = nc.dram_tensor("ff_b2", [LAYERS, 128, 4], F32,
                                   kind="ExternalInput")
    dram["hd_w1"] = nc.dram_tensor("hd_w1", [2, 128, 8, 512], F32,
                                   kind="ExternalInput")
    dram["hd_b1"] = nc.dram_tensor("hd_b1", [2, 128, 4], F32,
                                   kind="ExternalInput")
    dram["hd_w2"] = nc.dram_tensor("hd_w2", [2, 128, 4, 2], F32,
                                   kind="ExternalInput")
    dram["hd_b2"] = nc.dram_tensor("hd_b2", [1, 2, 2], F32,
                                   kind="ExternalInput")
    dram["ident"] = nc.dram_tensor("ident", [2, 128, 128], BF16,
                                   kind="ExternalInput")
    out_logits = nc.dram_tensor("logits", [1, 4], F32, kind="ExternalOutput")

    rg_pairs = [[0, 1], [2, 3], [4, 5], [6, 7]]

    with tile.TileContext(nc) as tc:
        with (
            nc.allow_low_precision(
                reason="deliberate fp8/bf16 activation pipeline"),
            tc.tile_pool(name="act", bufs=1) as act,
            tc.tile_pool(name="w", bufs=4) as wpool,
            tc.tile_pool(name="vext", bufs=1) as vpool,
            tc.tile_pool(name="consts", bufs=1) as cpool,
            tc.tile_pool(name="x", bufs=3) as xpool,
            tc.tile_pool(name="pbig", bufs=2, space="PSUM") as pbig,
            tc.tile_pool(name="pctx", bufs=2, space="PSUM") as pctx,
            tc.tile_pool(name="dram", bufs=1, space="DRAM") as dpool,
        ):
            # ---- constants ----
            ones_row = cpool.tile([1, 128], BF16, tag="ones_row")
            nc.vector.memset(ones_row[:], 1.0)
            oavg_bf = cpool.tile([128, 128], BF16, tag="oavg_bf")
            nc.vector.memset(oavg_bf[:], 1.0 / D)
            eps_col = cpool.tile([128, 1], F32, tag="eps_col")
            nc.vector.memset(eps_col[:], EPS)
            ident64 = cpool.tile([128, 128], BF16, tag="ident64")
            nc.sync.dma_start(ident64[:], dram["ident"][0])
            ident4k = cpool.tile([128, 128], BF16, tag="ident4k")
            nc.sync.dma_start(ident4k[:], dram["ident"][1])
            lng_sb = cpool.tile([128, LAYERS, 3, 4], F32, tag="lng")
            nc.sync.dma_start(lng_sb[:], dram["ln_g"][:])
            lnb_sb = cpool.tile([128, LAYERS, 3, 4], F32, tag="lnb")
            nc.sync.dma_start(lnb_sb[:], dram["ln_b"][:])
            consts = dict(ones_row=ones_row, oavg_bf=oavg_bf,
                          eps_col=eps_col, ident64=ident64, ident4k=ident4k)

            pools = dict(act=act, w=wpool, vext=vpool, consts=consts,
                         pbig=pbig, pctx=pctx, xpool=xpool,
                         lng_sb=[lng_sb[:, li] for li in range(LAYERS)],
                         lnb_sb=[lnb_sb[:, li] for li in range(LAYERS)])

            # ---- layer 0 input projection: x0T = w_in^T @ wristT + b + peT
            peT_sb = xpool.tile([128, DC, S], BF16, tag="x")
            nc.sync.dma_start(peT_sb[:], dram["peT"][:])
            wT_sb = []
            win_sb = []
            for chn in range(2):
                wt = act.tile([IN, S], F32R, tag=f"wT{chn}",
                              name=f"wT{chn}")
                nc.sync.dma_start(wt[:], dram["wT"][chn])
                wT_sb.append(wt)
                wi = act.tile([IN, D], F32R, tag=f"win{chn}",
                              name=f"win{chn}")
                nc.sync.dma_start(wi[:], dram["w_in"][chn])
                win_sb.append(wi)
            bin_sb = act.tile([128, 2, DC], F32, tag="bin")
            nc.sync.dma_start(bin_sb[:], dram["b_in"][:])

            # own x0 (bf16 + fp8) and the partner's x0 (fp8 only) computed
            # locally -- the wrist inputs are tiny, so duplicating the input
            # projection beats a collective for the first cross-attention.
            x_own = xpool.tile([128, DC, 1024], BF16, tag="x")
            x8h = [xpool.tile([128, DC, 512], FP8, tag=f"x8h{h}",
                              name=f"x8h{h}")
                   for h in range(NQ)]
            xp8_0 = act.tile([128, NQ, DC, 512], FP8, tag="xp8")
            xtmp = act.tile([128, 1024], BF16, tag="xptmp", bufs=1)
            for mc in range(DC):
                ps = pbig.tile([128, 1024], F32, tag="big")
                for nq in range(NQ):
                    nc.tensor.matmul(ps[:, nq * 512:(nq + 1) * 512],
                                     win_sb[0][:, mc * 128:(mc + 1) * 128],
                                     wT_sb[0][:, nq * 512:(nq + 1) * 512],
                                     start=True, stop=True)
                nc.vector.scalar_tensor_tensor(out=x_own[:, mc, :], in0=ps[:],
                                               scalar=bin_sb[:, 0, mc:mc + 1],
                                               in1=peT_sb[:, mc, :],
                                               op0=OP.add, op1=OP.add)
                for nq in range(NQ):
                    nc.gpsimd.tensor_copy(
                        out=x8h[nq][:, mc, :],
                        in_=x_own[:, mc, nq * 512:(nq + 1) * 512])
                ps = pbig.tile([128, 1024], F32, tag="big")
                for nq in range(NQ):
                    nc.tensor.matmul(ps[:, nq * 512:(nq + 1) * 512],
                                     win_sb[1][:, mc * 128:(mc + 1) * 128],
                                     wT_sb[1][:, nq * 512:(nq + 1) * 512],
                                     start=True, stop=True)
                nc.vector.scalar_tensor_tensor(out=xtmp[:], in0=ps[:],
                                               scalar=bin_sb[:, 1, mc:mc + 1],
                                               in1=peT_sb[:, mc, :],
                                               op0=OP.add, op1=OP.add)
                for nq in range(NQ):
                    nc.gpsimd.tensor_copy(
                        out=xp8_0[:, nq, mc, :],
                        in_=xtmp[:, nq * 512:(nq + 1) * 512])

            pid = nc.sync.partition_id()
            own_par = pid % 2
            partner_par = 1 - own_par

            # Pair-swap via ReduceScatter: own slot of agx_in is zeroed once,
            # x8 goes into the partner slot, and out = sum over ranks of the
            # own-slot lane = exactly the partner's x8.  One collective per
            # layer with output half the AllGather's (15us constant paid once).
            agx_in = dpool.tile([2, 128, NQ, DC, 512], FP8, tag="agxin")
            agx_out = dpool.tile([128, NQ, DC, 512], FP8, tag="agxout")
            zfill = act.tile([128, NQ, DC, 512], FP8, tag="zfill")
            nc.vector.memset(zfill[:], 0.0)
            nc.sync.dma_start(agx_in[ds(own_par, 1), :, :, :, :].opt(),
                              zfill[:])

            for li in range(n_layers):
                if li == 0:
                    xp8 = xp8_0
                else:
                    xp8 = act.tile([128, NQ, DC, 512], FP8, tag="xp8")
                    for ck in range(NQ):
                        nc.sync.dma_start(
                            agx_in[ds(partner_par, 1), :, ck, :, :].opt(),
                            x8h[ck][:])
                    nc.gpsimd.collective_compute(
                        "ReduceScatter", OP.add, replica_groups=rg_pairs,
                        ins=[agx_in.opt()], outs=[agx_out.opt()])
                    for ck in range(NQ):
                        nc.sync.dma_start(xp8[:, ck], agx_out[:, ck])
                xkv8 = [xp8[:, 0], xp8[:, 1]]

                lw = li % LAYERS
                xc, xc8 = _emit_attn(nc, pools, dram, x_own, x8h, xkv8, lw, 0,
                                     zb)
                xs, xs8 = _emit_attn(nc, pools, dram, xc, xc8, xc8, lw, 1, zb)
                x_own, x8h = _emit_ffn(nc, pools, dram, xs, xs8, lw, zb,
                                       need_x8=(li < n_layers - 1))

            # ---- mean pool over S -> pairwise allgather -> heads ----
            # prefetch head weights while the last layer still computes
            hb2 = act.tile([1, 2, 2], F32, tag="hb2")
            nc.sync.dma_start(hb2[:], dram["hd_b2"][:])
            hws = []
            for hd in range(2):
                hw1 = act.tile([128, 8, 512], F32, tag="hT", bufs=2,
                               name=f"hw1_{hd}")
                nc.sync.dma_start(hw1[:], dram["hd_w1"][hd])
                hw2 = act.tile([128, 4, 2], F32, tag="hw2", bufs=2,
                               name=f"hw2_{hd}")
                nc.sync.dma_start(hw2[:], dram["hd_w2"][hd])
                hb1 = act.tile([128, 4], F32, tag="hb1", bufs=2,
                               name=f"hb1_{hd}")
                nc.sync.dma_start(hb1[:], dram["hd_b1"][hd])
                hws.append((hw1, hw2, hb1))

            # per-half partial sums so pooling overlaps the last LayerNorm
            mean_h = act.tile([128, NQ, DC, 1], F32, tag="meanh")
            for nq in range(NQ):
                for dc in range(DC):
                    nc.vector.tensor_reduce(out=mean_h[:, nq, dc, :],
                                            in_=x_own[:, dc,
                                                      nq * 512:(nq + 1) * 512],
                                            axis=mybir.AxisListType.X,
                                            op=OP.add)
            mean_sb = act.tile([128, DC, 1], F32, tag="mean")
            for dc in range(DC):
                nc.vector.tensor_tensor(out=mean_sb[:, dc, :],
                                        in0=mean_h[:, 0, dc, :],
                                        in1=mean_h[:, 1, dc, :], op=OP.add)
            mb_in = dpool.tile([DC, 128, 1], F32, tag="mbin")
            for dc in range(DC):
                nc.sync.dma_start(mb_in[dc], mean_sb[:, dc, :])
            mb_out = dpool.tile([2 * DC, 128, 1], F32, tag="mbout")
            nc.gpsimd.collective_compute(
                "AllGather", OP.bypass, replica_groups=rg_pairs,
                ins=[mb_in.opt()], outs=[mb_out.opt()])
            fusedT = act.tile([128, 2 * DC, 1], F32, tag="fusedT")
            for kc in range(2 * DC):
                nc.sync.dma_start(fusedT[:, kc, :], mb_out[kc])

            logits_sb = act.tile([1, 4], F32, tag="logits")
            for hd in range(2):
                hw1, hw2, hb1 = hws[hd]
                o1 = act.tile([128, 4, 1], F32, tag="o1", bufs=2)
                for mc in range(DC):
                    ps = pbig.tile([128, 1024], F32, tag="big")
                    for kc in range(2 * DC):
                        nc.tensor.matmul(
                            ps[:, 0:1],
                            hw1[:, kc, mc * 128:(mc + 1) * 128],
                            fusedT[:, kc, :],
                            start=(kc == 0), stop=(kc == 2 * DC - 1))
                    nc.vector.tensor_scalar(out=o1[:, mc, :], in0=ps[:, 0:1],
                                            scalar1=hb1[:, mc:mc + 1],
                                            scalar2=0.0, op0=OP.add,
                                            op1=OP.max)
                lp = pbig.tile([128, 1024], F32, tag="big")
                for kc in range(DC):
                    nc.tensor.matmul(lp[0:1, 0:2], o1[:, kc, :],
                                     hw2[:, kc, :],
                                     start=(kc == 0), stop=(kc == DC - 1))
                nc.vector.tensor_tensor(out=logits_sb[0:1, hd * 2:hd * 2 + 2],
                                        in0=lp[0:1, 0:2], in1=hb2[0:1, hd, :],
                                        op=OP.add)
            nc.sync.dma_start(out_logits[:], logits_sb[:])

    nc.compile()
    return nc


def _prep(inputs):
    f32 = np.float32

    def g(k):
        return np.asarray(inputs[k], f32)

    lw, rw = g("left_wrist"), g("right_wrist")
    Wl, bl, Wr, br, pe = g("Wl"), g("bl"), g("Wr"), g("br"), g("pe")
    mha_w, mha_b = g("mha_w"), g("mha_b")
    mha_ln_g, mha_ln_b = g("mha_ln_g"), g("mha_ln_b")
    ff_w1, ff_b1, ff_w2, ff_b2 = g("ff_w1"), g("ff_b1"), g("ff_w2"), g("ff_b2")
    ff_ln_g, ff_ln_b = g("ff_ln_g"), g("ff_ln_b")
    h_w1 = [g("h1_w1"), g("h2_w1")]
    h_b1 = [g("h1_b1"), g("h2_b1")]
    h_w2 = [g("h1_w2"), g("h2_w2")]
    h_b2 = [g("h1_b2"), g("h2_b2")]

    peT = np.ascontiguousarray(
        pe.T.reshape(DC, 128, S).transpose(1, 0, 2)).astype(BF)

    def q8(x):
        return np.clip(x * WS, -240.0, 240.0).astype(F8)

    per_ch = {}
    for ch in range(2):
        blocks = (0, 2) if ch == 0 else (1, 3)
        qkv = np.zeros((LAYERS, 2, 128, 16, 512), F8)
        qkob = np.zeros((LAYERS, 2, 128, 3, 4), f32)
        vb = np.zeros((LAYERS, 2, 1, 512), BF)
        lng = np.zeros((128, LAYERS, 3, 4), f32)
        lnb = np.zeros((128, LAYERS, 3, 4), f32)
        fw1 = np.zeros((LAYERS, 128, 4, 2048), F8)
        fb1 = np.zeros((LAYERS, 128, 16), f32)
        fw2 = np.zeros((LAYERS, 128, 16, 512), F8)
        fb2 = np.zeros((LAYERS, 128, 4), f32)
        for li in range(LAYERS):
            for bi, blk in enumerate(blocks):
                for pi in range(3):  # q, k, v
                    qkv[li, bi, :, pi * 4:(pi + 1) * 4, :] = \
                        q8(mha_w[li, blk, pi]).reshape(DC, 128, D) \
                        .transpose(1, 0, 2)
                qkv[li, bi, :, 12:16, :] = q8(mha_w[li, blk, 3]) \
                    .reshape(DC, 128, D).transpose(1, 0, 2)
                # q/k biases at 64x (outputs held scaled); o bias natural
                for ci, pi, sc in ((0, 0, WS), (1, 1, WS), (2, 3, 1.0)):
                    qkob[li, bi, :, ci, :] = \
                        (sc * mha_b[li, blk, pi]).reshape(DC, 128).T
                vb[li, bi, 0] = (WS * mha_b[li, blk, 2]).astype(BF)
                lng[:, li, bi, :] = mha_ln_g[li, blk].reshape(DC, 128).T
                lnb[:, li, bi, :] = mha_ln_b[li, blk].reshape(DC, 128).T
            lng[:, li, 2, :] = ff_ln_g[li, ch].reshape(DC, 128).T
            lnb[:, li, 2, :] = ff_ln_b[li, ch].reshape(DC, 128).T
            fw1[li] = q8(ff_w1[li, ch]).reshape(DC, 128, F).transpose(1, 0, 2)
            fb1[li] = (WS * ff_b1[li, ch]).reshape(FC, 128).T
            fw2[li] = q8(ff_w2[li, ch]).reshape(FC, 128, D).transpose(1, 0, 2)
            fb2[li] = ff_b2[li, ch].reshape(DC, 128).T
        per_ch[ch] = dict(qkv_w=qkv, qko_b=qkob, v_b=vb,
                          ln_g=lng, ln_b=lnb, ff_w1=fw1, ff_b1=fb1,
                          ff_w2=fw2, ff_b2=fb2)

    hd_w1 = np.stack([(w / float(S)).reshape(2 * DC, 128, D)
                      .transpose(1, 0, 2) for w in h_w1]).astype(f32)
    hd_b1 = np.stack([b.reshape(DC, 128).T for b in h_b1]).astype(f32)
    hd_w2 = np.stack([w.reshape(DC, 128, 2).transpose(1, 0, 2)
                      for w in h_w2]).astype(f32)
    hd_b2 = np.stack([b.reshape(1, 2) for b in h_b2]).transpose(1, 0, 2).astype(f32)
    eye = np.eye(128, dtype=f32)
    ident = np.ascontiguousarray(
        np.stack([WS * eye, WS * WS * eye]).astype(BF))

    in_maps = []
    for core in range(NCORES):
        b, ch = core // 2, core % 2
        wrist = lw[b] if ch == 0 else rw[b]
        w_in = Wl if ch == 0 else Wr
        b_in = (bl if ch == 0 else br).reshape(DC, 128).T
        m = {k: np.ascontiguousarray(v) for k, v in per_ch[ch].items()}
        wrist2 = rw[b] if ch == 0 else lw[b]
        w_in2 = Wr if ch == 0 else Wl
        b_in2 = (br if ch == 0 else bl).reshape(DC, 128).T
        m["wT"] = np.ascontiguousarray(np.stack([wrist.T, wrist2.T]))
        m["w_in"] = np.ascontiguousarray(np.stack([w_in, w_in2]))
        m["b_in"] = np.ascontiguousarray(
            np.stack([b_in, b_in2], axis=1).astype(f32))
        m["peT"] = peT
        m["hd_w1"] = hd_w1
        m["hd_b1"] = hd_b1
        m["hd_w2"] = hd_w2
        m["hd_b2"] = hd_b2
        m["ident"] = ident
        in_maps.append(m)
    return in_maps


def run(inputs, trace=False, n_layers=LAYERS):
    zb = all(
        float(np.abs(np.asarray(inputs[k], np.float32)).max()) == 0.0
        for k in ("bl", "br", "mha_b", "ff_b1", "ff_b2", "mha_ln_b",
                  "ff_ln_b"))
    key = ("nc", n_layers, zb)
    if key not in _CACHE:
        _CACHE[key] = _build(n_layers, zb)
    nc = _CACHE[key]
    in_maps = _prep(inputs)
    res = run_bass_kernel_spmd(nc, in_maps, core_ids=list(range(NCORES)),
                               trace=trace)
    logits1 = np.zeros((B, 2), np.float32)
    logits2 = np.zeros((B, 2), np.float32)
    for b in range(B):
        out = res.results[2 * b]["logits"]
        logits1[b] = out[0, 0:2]
        logits2[b] = out[0, 2:4]
    return (logits1, logits2), res


def kernel(**inputs):
    out, _ = run(inputs, trace=False)
    return out


# revision 47
# speedup vs baseline: 1.0032x; 1.0032x over previous
"""Trainium2 Bass kernel for nn_DualChannelTransformer.

Sharding: 8 cores = 4 batches x 2 channels (left/right). Each core runs one
channel's transformer stack. Cross-attention K/V activations are swapped
between channel pairs once per layer with a single ReduceScatter (own slot
zeroed, fp8 payload, output = partner's x only -- half an AllGather's
traffic and one 15us collective constant instead of two). Layer 0 needs no
collective: each core also computes the partner channel's input projection
locally (the wrist inputs are tiny).

Compute layout: activations transposed (xT [D, S]); heavy matmuls run in
fp8e4 with DoubleRow perf mode (2 contraction tiles of 128 per instruction).
Weights are pre-scaled x64 so fp8 quantization stays in the normal range;
descales fold into evacuation ops / the exp scale. Residual adds ride the PE
via scaled-identity matmuls so PSUM evacuations stay single-op ScalarE
Copy/Relu activations (zero-bias fast path; a general biased path is kept
for robustness). Attention: scoresT [k, q] in bf16 (Q/K held at 64x scale),
exp on ScalarE with scale 1/(8*4096) writing fp8, fused ctx+denominator
DoubleRow matmul over st-pairs (V8 with a ones column, M=65). LayerNorm over
D uses all-ones bf16 matmuls for mean/E[x^2], Abs_reciprocal_sqrt for the
inverse std, and dual-emits bf16 x (gpsimd) + fp8 x8 (DVE) so the fp8
consumers are unblocked first.
"""

import os
import sys

import numpy as np

for _p in ("/opt/trn_rl_repo", "/root/.axon_site/_ro/trn_rl_repo"):
    if os.path.isdir(_p) and _p not in sys.path:
        sys.path.insert(0, _p)

import ml_dtypes

import concourse.bass as bass
import concourse.tile as tile
from concourse import bacc, mybir
from concourse.bass import ds
from concourse.bass_utils import run_bass_kernel_spmd

F32 = mybir.dt.float32
F32R = mybir.dt.float32r
BF16 = mybir.dt.bfloat16
FP8 = mybir.dt.float8e4
AF = mybir.ActivationFunctionType
OP = mybir.AluOpType
DR = mybir.MatmulPerfMode.DoubleRow
BF = ml_dtypes.bfloat16
F8 = ml_dtypes.float8_e4m3fn

B, S, IN, D, H, LAYERS, F = 4, 1024, 6, 512, 8, 4, 2048
DK = D // H
EPS = 1e-5
NCORES = 8
DC = D // 128   # 4 activation partition chunks
KP = DC // 2    # 2 kc pairs
FC = F // 128   # 16
NQ = S // 512   # 2 moving-dim chunks
KT8 = S // 128  # 8 k tiles
WS = 64.0       # fp8 weight scale
VP = 72         # padded DK+1 for vext (16B-aligned pair stride)

_CACHE = {}


def _emit_ln(nc, pools, a_t, g_sb, b_sb, ln_i, zb, need_x8=True):
    """LayerNorm over D (partition axis) of a_t [128,4,1024] bf16.

    Returns (x [128,4,1024] bf16, [x8h0, x8h1] fp8 half tiles [128,4,512]).
    """
    act, pbig, xpool, consts = (pools["act"], pools["pbig"], pools["xpool"],
                                pools["consts"])
    oavg_bf = consts["oavg_bf"]    # [128,128] bf16 = 1/512
    eps_col = consts["eps_col"]    # [128,1] f32 = EPS

    out = xpool.tile([128, DC, 1024], BF16, tag="x")
    x8h = [xpool.tile([128, DC, 512], FP8, tag=f"x8h{h}", name=f"x8h{h}")
           for h in range(NQ)] if need_x8 else [None, None]
    for nq in range(NQ):
        s0 = nq * 512
        sq = act.tile([128, DC, 512], BF16, tag="sq", bufs=2)
        for kc in range(DC):
            nc.vector.tensor_tensor(out=sq[:, kc, :],
                                    in0=a_t[:, kc, s0:s0 + 512],
                                    in1=a_t[:, kc, s0:s0 + 512], op=OP.mult)
        mps = pbig.tile([128, 1024], F32, tag="big")
        for kc in range(DC):
            nc.tensor.matmul(mps[:, 0:512], oavg_bf[:],
                             a_t[:, kc, s0:s0 + 512],
                             start=(kc == 0), stop=(kc == DC - 1))
        for kc in range(DC):
            nc.tensor.matmul(mps[:, 512:1024], oavg_bf[:],
                             sq[:, kc, :],
                             start=(kc == 0), stop=(kc == DC - 1))
        m2 = act.tile([128, 512], BF16, tag="lnt", bufs=3)
        nc.scalar.activation(out=m2[:], in_=mps[:, 0:512], func=AF.Square)
        work = act.tile([128, 512], BF16, tag="lnt2", bufs=3)
        nc.vector.tensor_tensor(out=work[:], in0=mps[:, 512:1024], in1=m2[:],
                                op=OP.subtract)
        # inv_std in one ACT op (arg is var+eps > 0, so abs is a no-op)
        nc.scalar.activation(out=work[:], in_=work[:],
                             func=AF.Abs_reciprocal_sqrt, bias=eps_col[:])
        ams = []
        for dc in range(DC):
            am = act.tile([128, 512], BF16, tag="am", bufs=4, name="am")
            nc.vector.tensor_tensor(out=am[:], in0=a_t[:, dc, s0:s0 + 512],
                                    in1=mps[:, 0:512], op=OP.subtract)
            ams.append(am)
        for dc in range(DC):
            am = ams[dc]
            if zb:
                # dual-emit: fp8 shadow on DVE (feeds next block's matmuls
                # first), bf16 canonical on gpsimd (residual, needed later);
                # final layer (no x8 consumer) takes the fast DVE path only
                nc.vector.scalar_tensor_tensor(
                    out=(x8h[nq][:, dc, :] if need_x8
                         else out[:, dc, s0:s0 + 512]),
                    in0=am[:], scalar=g_sb[:, ln_i, dc:dc + 1], in1=work[:],
                    op0=OP.mult, op1=OP.mult)
                if need_x8:
                    gt = act.tile([128, 512], BF16, tag="gt", bufs=4)
                    nc.gpsimd.tensor_scalar_mul(
                        out=gt[:], in0=am[:],
                        scalar1=g_sb[:, ln_i, dc:dc + 1])
                    nc.gpsimd.tensor_tensor(out=out[:, dc, s0:s0 + 512],
                                            in0=gt[:], in1=work[:],
                                            op=OP.mult)
            else:
                nc.vector.scalar_tensor_tensor(
                    out=am[:], in0=am[:], scalar=g_sb[:, ln_i, dc:dc + 1],
                    in1=work[:], op0=OP.mult, op1=OP.mult)
                nc.gpsimd.tensor_scalar_add(out=out[:, dc, s0:s0 + 512],
                                            in0=am[:],
                                            scalar1=b_sb[:, ln_i, dc:dc + 1])
                if need_x8:
                    nc.gpsimd.tensor_copy(out=x8h[nq][:, dc, :],
                                          in_=out[:, dc, s0:s0 + 512])
    return out, x8h


def _emit_attn(nc, pools, dram, x_q, xq8, xkv8, li, bi, zb):
    """One attention block.

    x_q: [128,4,1024] bf16 (residual); xq8/xkv8: fp8 half-tile lists.
    Returns (x, x8h) from the post-LN.
    """
    act, pbig, pctx, consts = (pools["act"], pools["pbig"], pools["pctx"],
                               pools["consts"])
    ones_row = consts["ones_row"]      # [1,128] bf16 ones
    ident64 = consts["ident64"]        # [128,128] bf16 64*I

    wqkv = pools["w"].tile([128, 16, 512], FP8, tag="wbig")
    nc.scalar.dma_start(wqkv[:], dram["qkv_w"][li, bi])
    bqko = vb = None
    if not zb:
        bqko = act.tile([128, 3, 4], F32, tag="bqko", bufs=2)
        nc.scalar.dma_start(bqko[:], dram["qko_b"][li, bi])
        vb = act.tile([1, 512], BF16, tag="vb", bufs=2)
        nc.scalar.dma_start(vb[:], dram["v_b"][li, bi])

    # ---- Q/K projections (transposed, bf16 out held at 64x scale) ----
    QT = act.tile([128, DC, 1024], BF16, tag="qt")
    KT = act.tile([128, DC, 1024], BF16, tag="kt")
    for pi, (dst, src) in enumerate(((QT, xq8), (KT, xkv8))):
        for nq in range(NQ):
            for mc in range(DC):
                ps = pbig.tile([128, 1024], F32, tag="big")
                for kp in range(KP):
                    nc.tensor.matmul(
                        ps[:, 0:512],
                        wqkv[:, pi * 4 + 2 * kp:pi * 4 + 2 * kp + 2,
                             mc * 128:(mc + 1) * 128],
                        src[nq][:, 2 * kp:2 * kp + 2, :],
                        start=(kp == 0), stop=(kp == KP - 1), perf_mode=DR)
                if zb:
                    nc.scalar.activation(
                        out=dst[:, mc, nq * 512:(nq + 1) * 512],
                        in_=ps[:, 0:512], func=AF.Copy)
                else:
                    nc.vector.tensor_scalar_add(
                        out=dst[:, mc, nq * 512:(nq + 1) * 512],
                        in0=ps[:, 0:512], scalar1=bqko[:, pi, mc:mc + 1])

    # ---- V projection + ones column for the denominator ----
    vext = []
    for sp in range(KT8 // 2):
        vt = pools["vext"].tile([128, 2, H, VP], FP8, tag=f"v{sp}",
                                name=f"v{sp}")
        vext.append(vt)
    for st in range(KT8):
        sp, stl = st // 2, st % 2
        hh, loc = st // 4, st % 4
        ps = pbig.tile([128, 1024], F32, tag="big")
        for kp in range(KP):
            nc.tensor.matmul(ps[:, 0:512],
                             xkv8[hh][:, 2 * kp:2 * kp + 2,
                                      loc * 128:(loc + 1) * 128],
                             wqkv[:, 8 + 2 * kp:10 + 2 * kp, :],
                             start=(kp == 0),
                             stop=(zb and kp == KP - 1), perf_mode=DR)
        if not zb:
            nc.tensor.matmul(ps[:, 0:512], ones_row[:], vb[:],
                             start=False, stop=True)
        if zb:
            nc.scalar.activation(
                out=vext[sp][:, stl, :, 0:DK],
                in_=ps[:, 0:512].rearrange("p (h k) -> p h k", h=H),
                func=AF.Copy, scale=float(1.0 / WS))
        else:
            nc.vector.tensor_scalar_mul(
                out=vext[sp][:, stl, :, 0:DK],
                in0=ps[:, 0:512].rearrange("p (h k) -> p h k", h=H),
                scalar1=1.0 / WS)
        nc.vector.memset(vext[sp][:, stl, :, DK:DK + 1], 1.0)

    # ---- attention core: scoresT -> exp(fp8) -> fused ctx+denom (DR) ----
    ctxT8 = act.tile([128, DC, 1024], FP8, tag="ctxT")
    for h in range(H):
        hp, hr = h // 2, (h % 2) * 64
        cps = pctx.tile([DK + 1, 1024], F32, tag="ctx")
        for sp in range(KT8 // 2):
            exp8 = act.tile([128, 2, 1024], FP8, tag="exp", bufs=6)
            for stl in range(2):
                st = sp * 2 + stl
                sps = pbig.tile([128, 1024], F32, tag="big")
                for nq in range(NQ):
                    nc.tensor.matmul(sps[:, nq * 512:(nq + 1) * 512],
                                     KT[hr:hr + 64, hp, st * 128:(st + 1) * 128],
                                     QT[hr:hr + 64, hp, nq * 512:(nq + 1) * 512],
                                     start=True, stop=True)
                nc.scalar.activation(out=exp8[:, stl, :], in_=sps[:],
                                     func=AF.Exp,
                                     scale=float(1.0 / (np.sqrt(DK) * WS * WS)))
            for nq in range(NQ):
                nc.tensor.matmul(cps[:, nq * 512:(nq + 1) * 512],
                                 vext[sp][:, :, h, 0:DK + 1],
                                 exp8[:, :, nq * 512:(nq + 1) * 512],
                                 start=(sp == 0), stop=(sp == KT8 // 2 - 1),
                                 perf_mode=DR)
        rd = act.tile([1, 1024], BF16, tag="rd", bufs=2)
        nc.vector.reciprocal(out=rd[:], in_=cps[DK:DK + 1, :])
        rdb = act.tile([64, 1024], BF16, tag="rdb", bufs=2)
        nc.gpsimd.partition_broadcast(out_ap=rdb[:], in_ap=rd[:])
        nc.vector.tensor_tensor(out=ctxT8[hr:hr + 64, hp, :], in0=cps[0:DK, :],
                                in1=rdb[:], op=OP.mult)

    # ---- O projection (DR) + bias + residual via 64*I matmul ----
    a_t = act.tile([128, DC, 1024], BF16, tag="a", bufs=2)
    for mc in range(DC):
        ps = pbig.tile([128, 1024], F32, tag="big")
        for nq in range(NQ):
            for kp in range(KP):
                nc.tensor.matmul(ps[:, nq * 512:(nq + 1) * 512],
                                 wqkv[:, 12 + 2 * kp:14 + 2 * kp,
                                      mc * 128:(mc + 1) * 128],
                                 ctxT8[:, 2 * kp:2 * kp + 2,
                                       nq * 512:(nq + 1) * 512],
                                 start=(kp == 0), stop=False, perf_mode=DR)
            nc.tensor.matmul(ps[:, nq * 512:(nq + 1) * 512], ident64[:],
                             x_q[:, mc, nq * 512:(nq + 1) * 512],
                             start=False, stop=True)
        if zb:
            nc.scalar.activation(out=a_t[:, mc, :], in_=ps[:], func=AF.Copy,
                                 scale=float(1.0 / WS))
        else:
            nc.vector.tensor_scalar(out=a_t[:, mc, :], in0=ps[:],
                                    scalar1=1.0 / WS,
                                    scalar2=bqko[:, 2, mc:mc + 1],
                                    op0=OP.mult, op1=OP.add)
    return _emit_ln(nc, pools, a_t, pools["lng_sb"][li], pools["lnb_sb"][li],
                    bi, zb)


def _emit_ffn(nc, pools, dram, x_t, x8h, li, zb, need_x8=True):
    act, pbig = pools["act"], pools["pbig"]
    ident4k = pools["consts"]["ident4k"]   # [128,128] bf16 4096*I
    w1 = pools["w"].tile([128, 4, 2048], FP8, tag="wbig")
    nc.scalar.dma_start(w1[:], dram["ff_w1"][li])
    w2 = pools["w"].tile([128, 16, 512], FP8, tag="wbig")
    nc.scalar.dma_start(w2[:], dram["ff_w2"][li])
    b1 = b2 = None
    if not zb:
        b1 = act.tile([128, 16], F32, tag="b1", bufs=2)
        nc.scalar.dma_start(b1[:], dram["ff_b1"][li])
        b2 = act.tile([128, 4], F32, tag="b2", bufs=2)
        nc.scalar.dma_start(b2[:], dram["ff_b2"][li])

    a_t = act.tile([128, DC, 1024], BF16, tag="a", bufs=2)
    for half in range(2):
        s0 = half * 512
        hT8 = act.tile([128, 16, 512], FP8, tag="hT", bufs=2)
        for mf in range(FC):
            ps = pbig.tile([128, 1024], F32, tag="big")
            for kp in range(KP):
                nc.tensor.matmul(ps[:, 0:512],
                                 w1[:, 2 * kp:2 * kp + 2,
                                    mf * 128:(mf + 1) * 128],
                                 x8h[half][:, 2 * kp:2 * kp + 2, :],
                                 start=(kp == 0), stop=(kp == KP - 1),
                                 perf_mode=DR)
            if zb:
                nc.scalar.activation(out=hT8[:, mf, :], in_=ps[:, 0:512],
                                     func=AF.Relu)
            else:
                nc.vector.tensor_scalar(out=hT8[:, mf, :], in0=ps[:, 0:512],
                                        scalar1=b1[:, mf:mf + 1], scalar2=0.0,
                                        op0=OP.add, op1=OP.max)
        for mc in range(DC):
            ps = pbig.tile([128, 1024], F32, tag="big")
            for kp in range(FC // 2):
                nc.tensor.matmul(ps[:, 0:512],
                                 w2[:, 2 * kp:2 * kp + 2,
                                    mc * 128:(mc + 1) * 128],
                                 hT8[:, 2 * kp:2 * kp + 2, :],
                                 start=(kp == 0), stop=False, perf_mode=DR)
            nc.tensor.matmul(ps[:, 0:512], ident4k[:],
                             x_t[:, mc, s0:s0 + 512],
                             start=False, stop=True)
            if zb:
                nc.scalar.activation(out=a_t[:, mc, s0:s0 + 512],
                                     in_=ps[:, 0:512], func=AF.Copy,
                                     scale=float(1.0 / (WS * WS)))
            else:
                nc.vector.tensor_scalar(out=a_t[:, mc, s0:s0 + 512],
                                        in0=ps[:, 0:512],
                                        scalar1=1.0 / (WS * WS),
                                        scalar2=b2[:, mc:mc + 1],
                                        op0=OP.mult, op1=OP.add)
    return _emit_ln(nc, pools, a_t, pools["lng_sb"][li], pools["lnb_sb"][li],
                    2, zb, need_x8)


def _build(n_layers=LAYERS, zb=True):
    nc = bacc.Bacc("TRN2", target_bir_lowering=False, debug=False,
                   num_devices=NCORES)

    dram = {}
    dram["wT"] = nc.dram_tensor("wT", [2, IN, S], F32R, kind="ExternalInput")
    dram["w_in"] = nc.dram_tensor("w_in", [2, IN, D], F32R,
                                  kind="ExternalInput")
    dram["b_in"] = nc.dram_tensor("b_in", [2, 128, DC], F32,
                                  kind="ExternalInput")
    dram["peT"] = nc.dram_tensor("peT", [128, DC, S], BF16,
                                 kind="ExternalInput")
    dram["qkv_w"] = nc.dram_tensor("qkv_w", [LAYERS, 2, 128, 16, 512], FP8,
                                   kind="ExternalInput")
    dram["qko_b"] = nc.dram_tensor("qko_b", [LAYERS, 2, 128, 3, 4], F32,
                                   kind="ExternalInput")
    dram["v_b"] = nc.dram_tensor("v_b", [LAYERS, 2, 1, 512], BF16,
                                 kind="ExternalInput")
    dram["ln_g"] = nc.dram_tensor("ln_g", [128, LAYERS, 3, 4], F32,
                                  kind="ExternalInput")
    dram["ln_b"] = nc.dram_tensor("ln_b", [128, LAYERS, 3, 4], F32,
                                  kind="ExternalInput")
    dram["ff_w1"] = nc.dram_tensor("ff_w1", [LAYERS, 128, 4, 2048], FP8,
                                   kind="ExternalInput")
    dram["ff_b1"] = nc.dram_tensor("ff_b1", [LAYERS, 128, 16], F32,
                                   kind="ExternalInput")
    dram["ff_w2"] = nc.dram_tensor("ff_w2", [LAYERS, 128, 16, 512], FP8,
                                   kind="ExternalInput")
    dram["ff_b2"] = nc.dram_tensor("ff_b2", [LAYERS, 128, 4], F32,
                                   kind="ExternalInput")
    dram["hd_w1"] = nc.dram_tensor("hd_w1", [2, 128, 8, 512], F32,
                                   kind="ExternalInput")
    dram["hd_b1"] = nc.dram_tensor("hd_b1", [2, 128, 4], F32,
                                   kind="ExternalInput")
    dram["hd_w2"] = nc.dram_tensor("hd_w2", [2, 128, 4, 2], F32,
                                   kind="ExternalInput")
    dram["hd_b2"] = nc.dram_tensor("hd_b2", [1, 2, 2], F32,
                                   kind="ExternalInput")
    dram["ident"] = nc.dram_tensor("ident", [2, 128, 128], BF16,
                                   kind="ExternalInput")
    out_logits = nc.dram_tensor("logits", [1, 4], F32, kind="ExternalOutput")

    rg_pairs = [[0, 1], [2, 3], [4, 5], [6, 7]]

    with tile.TileContext(nc) as tc:
        with (
            nc.allow_low_precision(
                reason="deliberate fp8/bf16 activation pipeline"),
            tc.tile_pool(name="act", bufs=1) as act,
            tc.tile_pool(name="w", bufs=4) as wpool,
            tc.tile_pool(name="vext", bufs=1) as vpool,
            tc.tile_pool(name="consts", bufs=1) as cpool,
            tc.tile_pool(name="x", bufs=3) as xpool,
            tc.tile_pool(name="pbig", bufs=2, space="PSUM") as pbig,
            tc.tile_pool(name="pctx", bufs=2, space="PSUM") as pctx,
            tc.tile_pool(name="dram", bufs=1, space="DRAM") as dpool,
        ):
            # ---- constants ----
            ones_row = cpool.tile([1, 128], BF16, tag="ones_row")
            nc.vector.memset(ones_row[:], 1.0)
            oavg_bf = cpool.tile([128, 128], BF16, tag="oavg_bf")
            nc.vector.memset(oavg_bf[:], 1.0 / D)
            eps_col = cpool.tile([128, 1], F32, tag="eps_col")
            nc.vector.memset(eps_col[:], EPS)
            ident64 = cpool.tile([128, 128], BF16, tag="ident64")
            nc.sync.dma_start(ident64[:], dram["ident"][0])
            ident4k = cpool.tile([128, 128], BF16, tag="ident4k")
            nc.sync.dma_start(ident4k[:], dram["ident"][1])
            lng_sb = cpool.tile([128, LAYERS, 3, 4], F32, tag="lng")
            nc.sync.dma_start(lng_sb[:], dram["ln_g"][:])
            lnb_sb = cpool.tile([128, LAYERS, 3, 4], F32, tag="lnb")
            nc.sync.dma_start(lnb_sb[:], dram["ln_b"][:])
            consts = dict(ones_row=ones_row, oavg_bf=oavg_bf,
                          eps_col=eps_col, ident64=ident64, ident4k=ident4k)

            pools = dict(act=act, w=wpool, vext=vpool, consts=consts,
                         pbig=pbig, pctx=pctx, xpool=xpool,
                         lng_sb=[lng_sb[:, li] for li in range(LAYERS)],
                         lnb_sb=[lnb_sb[:, li] for li in range(LAYERS)])

            # ---- layer 0 input projection: x0T = w_in^T @ wristT + b + peT
            peT_sb = xpool.tile([128, DC, S], BF16, tag="x")
            nc.sync.dma_start(peT_sb[:], dram["peT"][:])
            wT_sb = []
            win_sb = []
            for chn in range(2):
                wt = act.tile([IN, S], F32R, tag=f"wT{chn}",
                              name=f"wT{chn}")
                nc.sync.dma_start(wt[:], dram["wT"][chn])
                wT_sb.append(wt)
                wi = act.tile([IN, D], F32R, tag=f"win{chn}",
                              name=f"win{chn}")
                nc.sync.dma_start(wi[:], dram["w_in"][chn])
                win_sb.append(wi)
            bin_sb = act.tile([128, 2, DC], F32, tag="bin")
            nc.sync.dma_start(bin_sb[:], dram["b_in"][:])

            # own x0 (bf16 + fp8) and the partner's x0 (fp8 only) computed
            # locally -- the wrist inputs are tiny, so duplicating the input
            # projection beats a collective for the first cross-attention.
            x_own = xpool.tile([128, DC, 1024], BF16, tag="x")
            x8h = [xpool.tile([128, DC, 512], FP8, tag=f"x8h{h}",
                              name=f"x8h{h}")
                   for h in range(NQ)]
            xp8_0 = act.tile([128, NQ, DC, 512], FP8, tag="xp8")
            xtmp = act.tile([128, 1024], BF16, tag="xptmp", bufs=1)
            for mc in range(DC):
                ps = pbig.tile([128, 1024], F32, tag="big")
                for nq in range(NQ):
                    nc.tensor.matmul(ps[:, nq * 512:(nq + 1) * 512],
                                     win_sb[0][:, mc * 128:(mc + 1) * 128],
                                     wT_sb[0][:, nq * 512:(nq + 1) * 512],
                                     start=True, stop=True)
                nc.vector.scalar_tensor_tensor(out=x_own[:, mc, :], in0=ps[:],
                                               scalar=bin_sb[:, 0, mc:mc + 1],
                                               in1=peT_sb[:, mc, :],
                                               op0=OP.add, op1=OP.add)
                for nq in range(NQ):
                    nc.gpsimd.tensor_copy(
                        out=x8h[nq][:, mc, :],
                        in_=x_own[:, mc, nq * 512:(nq + 1) * 512])
                ps = pbig.tile([128, 1024], F32, tag="big")
                for nq in range(NQ):
                    nc.tensor.matmul(ps[:, nq * 512:(nq + 1) * 512],
                                     win_sb[1][:, mc * 128:(mc + 1) * 128],
                                     wT_sb[1][:, nq * 512:(nq + 1) * 512],
                                     start=True, stop=True)
                nc.vector.scalar_tensor_tensor(out=xtmp[:], in0=ps[:],
                                               scalar=bin_sb[:, 1, mc:mc + 1],
                                               in1=peT_sb[:, mc, :],
                                               op0=OP.add, op1=OP.add)
                for nq in range(NQ):
                    nc.gpsimd.tensor_copy(
                        out=xp8_0[:, nq, mc, :],
                        in_=xtmp[:, nq * 512:(nq + 1) * 512])

            pid = nc.sync.partition_id()
            own_par = pid % 2
            partner_par = 1 - own_par

            # Pair-swap via ReduceScatter: own slot of agx_in is zeroed once,
            # x8 goes into the partner slot, and out = sum over ranks of the
            # own-slot lane = exactly the partner's x8.  One collective per
            # layer with output half the AllGather's (15us constant paid once).
            agx_in = dpool.tile([2, 128, NQ, DC, 512], FP8, tag="agxin")
            agx_out = dpool.tile([128, NQ, DC, 512], FP8, tag="agxout")
            zfill = act.tile([128, NQ, DC, 512], FP8, tag="zfill")
            nc.vector.memset(zfill[:], 0.0)
            nc.sync.dma_start(agx_in[ds(own_par, 1), :, :, :, :].opt(),
                              zfill[:])

            for li in range(n_layers):
                if li == 0:
                    xp8 = xp8_0
                else:
                    xp8 = act.tile([128, NQ, DC, 512], FP8, tag="xp8")
                    for ck in range(NQ):
                        nc.sync.dma_start(
                            agx_in[ds(partner_par, 1), :, ck, :, :].opt(),
                            x8h[ck][:])
                    nc.gpsimd.collective_compute(
                        "ReduceScatter", OP.add, replica_groups=rg_pairs,
                        ins=[agx_in.opt()], outs=[agx_out.opt()])
                    for ck in range(NQ):
                        nc.sync.dma_start(xp8[:, ck], agx_out[:, ck])
                xkv8 = [xp8[:, 0], xp8[:, 1]]

                lw = li % LAYERS
                xc, xc8 = _emit_attn(nc, pools, dram, x_own, x8h, xkv8, lw, 0,
                                     zb)
                xs, xs8 = _emit_attn(nc, pools, dram, xc, xc8, xc8, lw, 1, zb)
                x_own, x8h = _emit_ffn(nc, pools, dram, xs, xs8, lw, zb,
                                       need_x8=(li < n_layers - 1))

            # ---- mean pool over S -> pairwise allgather -> heads ----
            # prefetch head weights while the last layer still computes
            hb2 = act.tile([1, 2, 2], F32, tag="hb2")
            nc.sync.dma_start(hb2[:], dram["hd_b2"][:])
            hws = []
            for hd in range(2):
                hw1 = act.tile([128, 8, 512], F32, tag="hT", bufs=2,
                               name=f"hw1_{hd}")
                nc.sync.dma_start(hw1[:], dram["hd_w1"][hd])
                hw2 = act.tile([128, 4, 2], F32, tag="hw2", bufs=2,
                               name=f"hw2_{hd}")
                nc.sync.dma_start(hw2[:], dram["hd_w2"][hd])
                hb1 = act.tile([128, 4], F32, tag="hb1", bufs=2,
                               name=f"hb1_{hd}")
                nc.sync.dma_start(hb1[:], dram["hd_b1"][hd])
                hws.append((hw1, hw2, hb1))

            # per-half partial sums so pooling overlaps the last LayerNorm
            mean_h = act.tile([128, NQ, DC, 1], F32, tag="meanh")
            for nq in range(NQ):
                for dc in range(DC):
                    nc.vector.tensor_reduce(out=mean_h[:, nq, dc, :],
                                            in_=x_own[:, dc,
                                                      nq * 512:(nq + 1) * 512],
                                            axis=mybir.AxisListType.X,
                                            op=OP.add)
            mean_sb = act.tile([128, DC, 1], F32, tag="mean")
            for dc in range(DC):
                nc.vector.tensor_tensor(out=mean_sb[:, dc, :],
                                        in0=mean_h[:, 0, dc, :],
                                        in1=mean_h[:, 1, dc, :], op=OP.add)
            mb_in = dpool.tile([DC, 128, 1], F32, tag="mbin")
            for dc in range(DC):
                nc.sync.dma_start(mb_in[dc], mean_sb[:, dc, :])
            mb_out = dpool.tile([2 * DC, 128, 1], F32, tag="mbout")
            nc.gpsimd.collective_compute(
                "AllGather", OP.bypass, replica_groups=rg_pairs,
                ins=[mb_in.opt()], outs=[mb_out.opt()])
            fusedT = act.tile([128, 2 * DC, 1], F32, tag="fusedT")
            for kc in range(2 * DC):
                nc.sync.dma_start(fusedT[:, kc, :], mb_out[kc])

            logits_sb = act.tile([1, 4], F32, tag="logits")
            for hd in range(2):
                hw1, hw2, hb1 = hws[hd]
                o1 = act.tile([128, 4, 1], F32, tag="o1", bufs=2)
                for mc in range(DC):
                    ps = pbig.tile([128, 1024], F32, tag="big")
                    for kc in range(2 * DC):
                        nc.tensor.matmul(
                            ps[:, 0:1],
                            hw1[:, kc, mc * 128:(mc + 1) * 128],
                            fusedT[:, kc, :],
                            start=(kc == 0), stop=(kc == 2 * DC - 1))
                    nc.vector.tensor_scalar(out=o1[:, mc, :], in0=ps[:, 0:1],
                                            scalar1=hb1[:, mc:mc + 1],
                                            scalar2=0.0, op0=OP.add,
                                            op1=OP.max)
                lp = pbig.tile([128, 1024], F32, tag="big")
                for kc in range(DC):
                    nc.tensor.matmul(lp[0:1, 0:2], o1[:, kc, :],
                                     hw2[:, kc, :],
                                     start=(kc == 0), stop=(kc == DC - 1))
                nc.vector.tensor_tensor(out=logits_sb[0:1, hd * 2:hd * 2 + 2],
                                        in0=lp[0:1, 0:2], in1=hb2[0:1, hd, :],
                                        op=OP.add)
            nc.sync.dma_start(out_logits[:], logits_sb[:])

    nc.compile()
    return nc


def _prep(inputs):
    f32 = np.float32

    def g(k):
        return np.asarray(inputs[k], f32)

    lw, rw = g("left_wrist"), g("right_wrist")
    Wl, bl, Wr, br, pe = g("Wl"), g("bl"), g("Wr"), g("br"), g("pe")
    mha_w, mha_b = g("mha_w"), g("mha_b")
    mha_ln_g, mha_ln_b = g("mha_ln_g"), g("mha_ln_b")
    ff_w1, ff_b1, ff_w2, ff_b2 = g("ff_w1"), g("ff_b1"), g("ff_w2"), g("ff_b2")
    ff_ln_g, ff_ln_b = g("ff_ln_g"), g("ff_ln_b")
    h_w1 = [g("h1_w1"), g("h2_w1")]
    h_b1 = [g("h1_b1"), g("h2_b1")]
    h_w2 = [g("h1_w2"), g("h2_w2")]
    h_b2 = [g("h1_b2"), g("h2_b2")]

    peT = np.ascontiguousarray(
        pe.T.reshape(DC, 128, S).transpose(1, 0, 2)).astype(BF)

    def q8(x):
        return np.clip(x * WS, -240.0, 240.0).astype(F8)

    per_ch = {}
    for ch in range(2):
        blocks = (0, 2) if ch == 0 else (1, 3)
        qkv = np.zeros((LAYERS, 2, 128, 16, 512), F8)
        qkob = np.zeros((LAYERS, 2, 128, 3, 4), f32)
        vb = np.zeros((LAYERS, 2, 1, 512), BF)
        lng = np.zeros((128, LAYERS, 3, 4), f32)
        lnb = np.zeros((128, LAYERS, 3, 4), f32)
        fw1 = np.zeros((LAYERS, 128, 4, 2048), F8)
        fb1 = np.zeros((LAYERS, 128, 16), f32)
        fw2 = np.zeros((LAYERS, 128, 16, 512), F8)
        fb2 = np.zeros((LAYERS, 128, 4), f32)
        for li in range(LAYERS):
            for bi, blk in enumerate(blocks):
                for pi in range(3):  # q, k, v
                    qkv[li, bi, :, pi * 4:(pi + 1) * 4, :] = \
                        q8(mha_w[li, blk, pi]).reshape(DC, 128, D) \
                        .transpose(1, 0, 2)
                qkv[li, bi, :, 12:16, :] = q8(mha_w[li, blk, 3]) \
                    .reshape(DC, 128, D).transpose(1, 0, 2)
                # q/k biases at 64x (outputs held scaled); o bias natural
                for ci, pi, sc in ((0, 0, WS), (1, 1, WS), (2, 3, 1.0)):
                    qkob[li, bi, :, ci, :] = \
                        (sc * mha_b[li, blk, pi]).reshape(DC, 128).T
                vb[li, bi, 0] = (WS * mha_b[li, blk, 2]).astype(BF)
                lng[:, li, bi, :] = mha_ln_g[li, blk].reshape(DC, 128).T
                lnb[:, li, bi, :] = mha_ln_b[li, blk].reshape(DC, 128).T
            lng[:, li, 2, :] = ff_ln_g[li, ch].reshape(DC, 128).T
            lnb[:, li, 2, :] = ff_ln_b[li, ch].reshape(DC, 128).T
            fw1[li] = q8(ff_w1[li, ch]).reshape(DC, 128, F).transpose(1, 0, 2)
            fb1[li] = (WS * ff_b1[li, ch]).reshape(FC, 128).T
            fw2[li] = q8(ff_w2[li, ch]).reshape(FC, 128, D).transpose(1, 0, 2)
            fb2[li] = ff_b2[li, ch].reshape(DC, 128).T
        per_ch[ch] = dict(qkv_w=qkv, qko_b=qkob, v_b=vb,
                          ln_g=lng, ln_b=lnb, ff_w1=fw1, ff_b1=fb1,
                          ff_w2=fw2, ff_b2=fb2)

    hd_w1 = np.stack([(w / float(S)).reshape(2 * DC, 128, D)
                      .transpose(1, 0, 2) for w in h_w1]).astype(f32)
    hd_b1 = np.stack([b.reshape(DC, 128).T for b in h_b1]).astype(f32)
    hd_w2 = np.stack([w.reshape(DC, 128, 2).transpose(1, 0, 2)
                      for w in h_w2]).astype(f32)
    hd_b2 = np.stack([b.reshape(1, 2) for b in h_b2]).transpose(1, 0, 2).astype(f32)
    eye = np.eye(128, dtype=f32)
    ident = np.ascontiguousarray(
        np.stack([WS * eye, WS * WS * eye]).astype(BF))

    in_maps = []
    for core in range(NCORES):
        b, ch = core // 2, core % 2
        wrist = lw[b] if ch == 0 else rw[b]
        w_in = Wl if ch == 0 else Wr
        b_in = (bl if ch == 0 else br).reshape(DC, 128).T
        m = {k: np.ascontiguousarray(v) for k, v in per_ch[ch].items()}
        wrist2 = rw[b] if ch == 0 else lw[b]
        w_in2 = Wr if ch == 0 else Wl
        b_in2 = (br if ch == 0 else bl).reshape(DC, 128).T
        m["wT"] = np.ascontiguousarray(np.stack([wrist.T, wrist2.T]))
        m["w_in"] = np.ascontiguousarray(np.stack([w_in, w_in2]))
        m["b_in"] = np.ascontiguousarray(
            np.stack([b_in, b_in2], axis=1).astype(f32))
        m["peT"] = peT
        m["hd_w1"] = hd_w1
        m["hd_b1"] = hd_b1
        m["hd_w2"] = hd_w2
        m["hd_b2"] = hd_b2
        m["ident"] = ident
        in_maps.append(m)
    return in_maps


def run(inputs, trace=False, n_layers=LAYERS):
    zb = all(
        float(np.abs(np.asarray(inputs[k], np.float32)).max()) == 0.0
        for k in ("bl", "br", "mha_b", "ff_b1", "ff_b2", "mha_ln_b",
                  "ff_ln_b"))
    key = ("nc", n_layers, zb)
    if key not in _CACHE:
        _CACHE[key] = _build(n_layers, zb)
    nc = _CACHE[key]
    in_maps = _prep(inputs)
    res = run_bass_kernel_spmd(nc, in_maps, core_ids=list(range(NCORES)),
                               trace=trace)
    logits1 = np.zeros((B, 2), np.float32)
    logits2 = np.zeros((B, 2), np.float32)
    for b in range(B):
        out = res.results[2 * b]["logits"]
        logits1[b] = out[0, 0:2]
        logits2[b] = out[0, 2:4]
    return (logits1, logits2), res


def kernel(**inputs):
    out, _ = run(inputs, trace=False)
    return out
